# revision 48
# baseline (speedup 1.0000x reference)
"""Trainium2 Bass kernel for nn_ContrastiveLoss (B=4096, D=512, 8 cores).

Strategy (row-sharded, per the sharding hint):
  Each core owns 512 query rows.  It computes its row-blocks of the three
  similarity matrices S_vi, S_ii, S_vv as fp32 PE matmuls (lhsT = transposed
  local features, rhs = full transposed features with the *key axis rotated*
  per-core so that the same-identity column band sits at columns [0, W) for
  every core — this keeps the compiled program identical across cores).

  Per 128-row tile and matrix, the 4096-wide row lives in PSUM as four
  [128,1024] quarter tiles.  Each quarter gets its own row-max (DVE reduce,
  negated) so exp (ScalarE, fused row-sum via accum_out) can drain a quarter
  as soon as its own max is known — PSUM double-buffers, PE never stalls on
  the softmax tail.  Per-row fixup factors g_q = exp(m_q - max_q m_q)
  reconcile the per-quarter scales afterwards (cheap [128,4] ops).

  Masked (same-identity) sums only touch the W-wide band: mask built once
  per row tile from labels (is_equal against broadcast band labels), the
  band multiply runs on GpSimd, band row-sum on DVE.  The cross-core
  numerator of the i->v direction is a masked *column* sum: accumulated in
  SBUF across row tiles, reduced over partitions with a ones-vector PE
  matmul at the end, and all-reduced across cores on the host (it is the
  natural gather step — 8 x W floats).

  Host combine is fp32 in the reference's op order so fp32 degeneracies
  (underflow -> 0/0 -> NaN) reproduce faithfully.
"""

import sys

if "/opt/trn_rl_repo" not in sys.path:
    sys.path.insert(0, "/opt/trn_rl_repo")

from contextlib import ExitStack

import numpy as np

import concourse.bass as bass
import concourse.tile as tile
from concourse import mybir
from concourse.bass_utils import run_bass_kernel_spmd

F32 = mybir.dt.float32
BF16 = mybir.dt.bfloat16
FP16 = mybir.dt.float16
AX = mybir.AxisListType
ALU = mybir.AluOpType
ACTF = mybir.ActivationFunctionType

# "fp16": single-pass fp16 matmuls.  Final loss error ~2e-4 (per-row S error
#   ~1e-2 absolute averages down 64x in the 4096-row mean).
# "fp32": 3-pass hi/lo bf16 matmuls, S accurate to ~5e-5 (loss err ~1e-5) at
#   3x the PE cost.
PRECISION = "fp16"

B = 4096          # batch (rows of f_v / f_i)
D = 512           # feature dim
N_CORES = 8
RPC = B // N_CORES          # rows per core = 512
RT = RPC // 128             # row tiles per core = 4
KT = D // 128               # contraction k-tiles = 4
QW = 1024                   # PSUM quarter width (2 banks)
NQ = B // QW                # quarters per row = 4
CH = 512                    # matmul N-chunk (one PSUM bank)

_prog_cache: dict = {}


# --------------------------------------------------------------------------
# BIR legalization: this container's walrus encodes exactly one sem-wait and
# one sem-update per TPB instruction; Tile emits several.  Hoist extras onto
# adjacent single-wait/-update InstEventSemaphore instructions.
# --------------------------------------------------------------------------
_SPLIT_ID = [0]


def _legalize_syncs(nc, strip_final_barrier=True):
    if strip_final_barrier:
        # The Tile epilogue is: drain-all, all-engine barrier, semaphore
        # reset (InstISA on Pool), second all-engine barrier.  The second
        # barrier only orders engine halt vs nothing — execution completes
        # when all queues drain regardless, and the reset still runs before
        # the NEFF can be re-executed.  Dropping it saves ~3-4us of tail.
        for f in nc.m.functions:
            for blk in f.blocks:
                if not blk.name.endswith("_end"):
                    continue
                insts = list(blk.instructions)
                isa_idx = max((i for i, ins in enumerate(insts)
                               if type(ins).__name__ == "InstISA"), default=None)
                if isa_idx is not None and isa_idx < len(insts) - 1:
                    while len(blk.instructions) > isa_idx + 1:
                        blk.instructions.pop()
    for f in nc.m.functions:
        for blk in f.blocks:
            insts = list(blk.instructions)
            out = []
            changed = False
            for ins in insts:
                si = ins.sync_info
                if si is None:
                    out.append(ins)
                    continue
                waits = list(si.on_wait or [])
                updates = list(si.on_update or [])
                pre, post = [], []
                if len(waits) > 1:
                    changed = True
                    for w in waits[:-1]:
                        _SPLIT_ID[0] += 1
                        pre.append(mybir.InstEventSemaphore(
                            name=f"WSPLIT-{_SPLIT_ID[0]}", engine=ins.engine,
                            ins=[], outs=[],
                            sync_info=mybir.SyncInfo(on_wait=[w], on_update=[])))
                    waits = waits[-1:]
                if len(updates) > 1:
                    assert "DMA" not in type(ins).__name__, (
                        f"cannot split updates on DMA inst {ins.name}")
                    changed = True
                    for u in updates[1:]:
                        _SPLIT_ID[0] += 1
                        post.append(mybir.InstEventSemaphore(
                            name=f"USPLIT-{_SPLIT_ID[0]}", engine=ins.engine,
                            ins=[], outs=[],
                            sync_info=mybir.SyncInfo(on_wait=[], on_update=[u])))
                    updates = updates[:1]
                if pre or post:
                    ins.sync_info = mybir.SyncInfo(on_wait=waits, on_update=updates)
                out.extend(pre)
                out.append(ins)
                out.extend(post)
            if changed:
                while len(blk.instructions):
                    blk.instructions.pop()
                for ins in out:
                    blk.instructions.append(ins)


# --------------------------------------------------------------------------
# Device program
# --------------------------------------------------------------------------
def build_program(W: int, legalize: bool = True, precision: str = PRECISION) -> bass.Bass:
    """One SPMD program, identical across cores; W = masked band width."""
    assert W <= QW and W % 128 == 0 and W >= CH
    nc = bass.Bass()

    # fp32 PE matmuls lower to two half-rate passes on TRN2 (FP32HI/LO) —
    # 4x the cost of 16-bit.  Use 16-bit operands instead: either a single
    # fp16 pass, or a 3-pass hi/lo bf16 split (fp32-level accuracy).
    if precision == "fp16":
        parts, mm_dt = ("hi",), FP16
    else:
        parts, mm_dt = ("hi", "lo"), BF16
    feat = {}
    for nm in ("lhs_v", "lhs_i", "rhs_i", "rhs_v"):
        shape = [D, RPC] if nm.startswith("lhs") else [D, B]
        feat[nm] = tuple(
            nc.declare_dram_parameter(f"{nm}_{p}", shape, mm_dt, isOutput=False)
            for p in parts)
    lab_loc = nc.declare_dram_parameter("lab_loc", [RPC], F32, isOutput=False)
    lab_band = nc.declare_dram_parameter("lab_band", [1, W], F32, isOutput=False)
    stats = nc.declare_dram_parameter("stats", [128, RT * 5], F32, isOutput=True)
    colsum = nc.declare_dram_parameter("colsum", [1, W], F32, isOutput=True)

    with ExitStack() as ctx:
        tc = ctx.enter_context(tile.TileContext(nc))
        const = ctx.enter_context(tc.tile_pool(name="const", bufs=1))
        lhsp = ctx.enter_context(tc.tile_pool(name="lhsp", bufs=1))
        rhsp = ctx.enter_context(tc.tile_pool(name="rhsp", bufs=1))
        ebandp = ctx.enter_context(tc.tile_pool(name="ebandp", bufs=3))
        scrp = ctx.enter_context(tc.tile_pool(name="scrp", bufs=2))
        mbandp = ctx.enter_context(tc.tile_pool(name="mbandp", bufs=2))
        smallp = ctx.enter_context(tc.tile_pool(name="smallp", bufs=4))
        outp = ctx.enter_context(tc.tile_pool(name="outp", bufs=1))
        psum = ctx.enter_context(tc.tile_pool(name="psum", bufs=4, space="PSUM"))

        # ---- feature tiles (DMA order = first-needed first) ----
        # 1) lhs_v + the first column group of rhs_i gate the very first
        #    matmul; 2) labels/masks are needed ~25us in; 3) the rest.
        lhs_sb = {}
        rhs_sb = {}

        def lhs_dma(key, per_kt=False):
            pair = []
            for pi, pn in enumerate(parts):
                t_ = lhsp.tile([128, KT, RPC], mm_dt, tag=f"lhs{key}{pn}",
                               name=f"lhs{key}{pn}")
                pair.append(t_)
                src = feat[f"lhs_{key}"][pi][:, :].rearrange("(kt p) m -> p kt m", p=128)
                if per_kt:
                    for kt in range(KT):
                        nc.sync.dma_start(out=t_[:, kt, :], in_=src[:, kt, :])
                else:
                    nc.sync.dma_start(out=t_, in_=src)
            lhs_sb[key] = pair

        def rhs_alloc(key):
            rhs_sb[key] = [rhsp.tile([128, KT, B], mm_dt, tag=f"rhs{key}{pn}",
                                     name=f"rhs{key}{pn}")
                           for pn in parts]

        def rhs_dma(key, cg, kts=None):
            for pi in range(len(parts)):
                dram = feat[f"rhs_{key}"][pi]
                t_ = rhs_sb[key][pi]
                for kt in (range(KT) if kts is None else kts):
                    nc.sync.dma_start(
                        out=t_[:, kt, cg * 1024:(cg + 1) * 1024],
                        in_=dram[kt * 128:(kt + 1) * 128, cg * 1024:(cg + 1) * 1024])

        rhs_alloc("i")
        rhs_alloc("v")
        # ramp-critical loads at [128, 512] granularity, kt-interleaved, so
        # the very first matmul (kt0, chunk 0 of quarter 0) gates on ~0.25MB
        lhs_v_tiles = [lhsp.tile([128, KT, RPC], mm_dt, tag=f"lhsv{pn}",
                                 name=f"lhsv{pn}") for pn in parts]
        lhs_sb["v"] = lhs_v_tiles
        lhs_src = [feat["lhs_v"][pi][:, :].rearrange("(kt p) m -> p kt m", p=128)
                   for pi in range(len(parts))]
        for kt in range(KT):
            for pi in range(len(parts)):
                nc.sync.dma_start(out=lhs_v_tiles[pi][:, kt, :],
                                  in_=lhs_src[pi][:, kt, :])
                dram = feat["rhs_i"][pi]
                t_ = rhs_sb["i"][pi]
                for cc in range(2):
                    nc.sync.dma_start(
                        out=t_[:, kt, cc * 512:(cc + 1) * 512],
                        in_=dram[kt * 128:(kt + 1) * 128, cc * 512:(cc + 1) * 512])

        rhs_dma("i", 1)

        lab_loc_sb = const.tile([128, RT], F32)
        nc.sync.dma_start(out=lab_loc_sb,
                          in_=lab_loc[:].rearrange("(t p) -> p t", p=128))
        lab_band_bc = const.tile([128, W], F32)
        lb = lab_band[:, :]
        nc.sync.dma_start(
            out=lab_band_bc,
            in_=bass.AP(tensor=lb.tensor, offset=lb.offset, ap=[[0, 128]] + list(lb.ap)[1:]),
        )

        rhs_dma("i", 2)
        rhs_dma("i", 3)

        ones = const.tile([128, 1], BF16)
        nc.vector.memset(ones, 1.0)

        # NB: tensor_scalar with an AP scalar lowers to TensorScalarPtr,
        # which measures ~9.5us per op on this silicon — use tensor_tensor
        # with stride-0 broadcast APs instead everywhere.
        masks = []
        for t in range(RT):
            m = const.tile([128, W], BF16, tag=f"mask{t}")
            masks.append(m)
            nc.vector.tensor_tensor(out=m, in0=lab_band_bc,
                                    in1=lab_loc_sb[:, t:t + 1].broadcast_to((128, W)),
                                    op=ALU.is_equal)

        lhs_dma("i")
        for cg in range(4):
            rhs_dma("v", cg)

        # ---- outputs / accumulators ----
        outstats = outp.tile([128, RT * 5], F32)
        macc = outp.tile([128, W], BF16)
        colsum_sb = outp.tile([1, W], F32)

        # exp segment layout: one ScalarE activation per (quarter x band/scr
        # region).  ScalarE reads up to the full [128, QW] PSUM quarter in one
        # op; the only split points are the band edge W (different dst) and a
        # 512 cap on band writes into e_band.
        seg_list = []          # (quarter, kind, lo, hi, accum_col)
        acc_col = 0
        for q in range(NQ):
            qlo, qhi = q * QW, (q + 1) * QW
            bounds = sorted({qlo, qhi, min(max(W, qlo), qhi)})
            for lo, hi in zip(bounds[:-1], bounds[1:]):
                kind = "band" if hi <= W else "scr"
                seg_list.append((q, kind, lo, hi, acc_col))
                acc_col += 1
        n_acc = acc_col
        # accum columns per quarter (for row-sum reconstruction)
        q_cols = [[s[4] for s in seg_list if s[0] == q] for q in range(NQ)]

        phases = (("vi", "v", "i"), ("ii", "i", "i"), ("vv", "v", "v"))
        for mname, lk, rk in phases:
            for t in range(RT):
                nh = smallp.tile([128, NQ], F32, tag="nh")
                rs = smallp.tile([128, n_acc], F32, tag="rs")
                e_band = ebandp.tile([128, W], BF16, tag="eband")
                ps_tiles = []
                # hi/lo passes; (hi,hi),(hi,lo) share the stationary
                # operand so weight loads amortize
                passes = ((0, 0),) if precision == "fp16" else ((0, 0), (0, 1), (1, 0))
                for q in range(NQ):
                    ps = psum.tile([128, QW], F32, tag="ps")
                    ps_tiles.append(ps)
                    for kt in range(KT):
                        for pidx, (pl, pr) in enumerate(passes):
                            for cq in range(QW // CH):
                                c = q * (QW // CH) + cq
                                nc.tensor.matmul(
                                    ps[:, cq * CH:(cq + 1) * CH],
                                    lhsT=lhs_sb[lk][pl][:, kt, t * 128:(t + 1) * 128],
                                    rhs=rhs_sb[rk][pr][:, kt, c * CH:(c + 1) * CH],
                                    start=(kt == 0 and pidx == 0),
                                    stop=(kt == KT - 1 and pidx == len(passes) - 1))
                    # negated quarter row-max
                    nc.vector.tensor_reduce(out=nh[:, q:q + 1], in_=ps, axis=AX.X,
                                            op=ALU.max, negate=True)

                # fixup factors g_q = exp(m_q - m),  nh = -m_q
                # gtmp = nh - mmneg = m - m_q;  g4 = exp(-gtmp)
                mmneg = smallp.tile([128, 1], F32, tag="mmneg")
                nc.vector.tensor_reduce(out=mmneg, in_=nh, axis=AX.X, op=ALU.min)
                gtmp = smallp.tile([128, NQ], F32, tag="gtmp")
                nc.vector.tensor_tensor(out=gtmp, in0=nh,
                                        in1=mmneg.broadcast_to((128, NQ)),
                                        op=ALU.subtract)
                g4 = smallp.tile([128, NQ], F32, tag="g4")
                nc.scalar.activation(out=g4, in_=gtmp, func=ACTF.Exp, scale=-1.0)

                # exp + fused row-sums, draining PSUM quarter by quarter
                for q, kind, lo, hi, acol in seg_list:
                    src = ps_tiles[q][:, lo - q * QW:hi - q * QW]
                    if kind == "band":
                        dst = e_band[:, lo:hi]
                    else:
                        scr = scrp.tile([128, QW], F32, tag="scr")
                        dst = scr[:, :hi - lo]
                    nc.scalar.activation(out=dst, in_=src, func=ACTF.Exp,
                                         bias=nh[:, q:q + 1], scale=1.0,
                                         accum_out=rs[:, acol:acol + 1])

                # full row-sum (not needed for the vi matrix):
                # sum_q g_q * (sum of that quarter's segment row-sums)
                if mname != "vi":
                    gr = smallp.tile([128, NQ], F32, tag="gr")
                    # W <= QW so only quarter 0 can have two segments
                    c0 = q_cols[0]
                    if len(c0) > 1:
                        nc.vector.tensor_reduce(out=gr[:, 0:1],
                                                in_=rs[:, c0[0]:c0[-1] + 1],
                                                axis=AX.X, op=ALU.add)
                        nc.vector.tensor_tensor(out=gr[:, 0:1], in0=gr[:, 0:1],
                                                in1=g4[:, 0:1], op=ALU.mult)
                        nc.vector.tensor_tensor(out=gr[:, 1:NQ], in0=g4[:, 1:NQ],
                                                in1=rs[:, c0[-1] + 1:n_acc], op=ALU.mult)
                    else:
                        nc.vector.tensor_tensor(out=gr, in0=g4, in1=rs[:, 0:n_acc],
                                                op=ALU.mult)
                    rs_col = {"ii": 3, "vv": 1}[mname]
                    nc.vector.tensor_reduce(out=outstats[:, t * 5 + rs_col:t * 5 + rs_col + 1],
                                            in_=gr, axis=AX.X, op=ALU.add)

                # masked band sums (band lies in quarter 0 -> scale by g4[:,0])
                masked = mbandp.tile([128, W], BF16, tag="masked")
                nc.vector.tensor_tensor(out=masked, in0=e_band, in1=masks[t], op=ALU.mult)
                praw = smallp.tile([128, 1], F32, tag="praw")
                nc.vector.tensor_reduce(out=praw, in_=masked, axis=AX.X, op=ALU.add)
                ps_col = {"vi": 0, "vv": 2, "ii": 4}[mname]
                nc.vector.tensor_tensor(out=outstats[:, t * 5 + ps_col:t * 5 + ps_col + 1],
                                        in0=praw, in1=g4[:, 0:1], op=ALU.mult)

                if mname == "vi":
                    # accumulate final-scale masked band for the column sums
                    g0b = g4[:, 0:1].broadcast_to((128, W))
                    if t == 0:
                        nc.vector.tensor_tensor(out=macc, in0=masked, in1=g0b,
                                                op=ALU.mult)
                    else:
                        msc = mbandp.tile([128, W], BF16, tag="msc")
                        nc.vector.tensor_tensor(out=msc, in0=masked, in1=g0b,
                                                op=ALU.mult)
                        nc.vector.tensor_tensor(out=macc, in0=macc, in1=msc, op=ALU.add)

        # ---- column-sum over the 512 local rows via ones-vector matmuls ----
        cps = psum.tile([128, QW], F32, tag="ps")
        nc.tensor.matmul(cps[0:1, 0:CH], lhsT=ones, rhs=macc[:, 0:CH],
                         start=True, stop=True)
        if W > CH:
            nc.tensor.matmul(cps[0:1, CH:W], lhsT=ones, rhs=macc[:, CH:W],
                             start=True, stop=True)
        nc.scalar.copy(out=colsum_sb, in_=cps[0:1, 0:W])
        nc.sync.dma_start(out=colsum[:, :], in_=colsum_sb)
        nc.sync.dma_start(out=stats[:, :], in_=outstats)

    if legalize:
        _legalize_syncs(nc)
    return nc


def get_program(W: int, legalize: bool = True, precision: str = PRECISION) -> bass.Bass:
    key = (W, legalize, precision)
    if key not in _prog_cache:
        _prog_cache[key] = build_program(W, legalize=legalize, precision=precision)
    return _prog_cache[key]


# --------------------------------------------------------------------------
# Host side
# --------------------------------------------------------------------------
def _hi_lo(x):
    """bf16 hi/lo split: x == hi + lo to ~2^-18 relative."""
    import ml_dtypes
    bf = ml_dtypes.bfloat16
    hi = x.astype(bf)
    lo = (x - hi.astype(np.float32)).astype(bf)
    return hi, lo


def prepare_inputs(f_v, f_i, labels, W, precision: str = PRECISION):
    """Returns (in_maps, c_lo) for the 8 cores."""
    fTv = np.ascontiguousarray(f_v.T.astype(np.float32, copy=False))
    fTi = np.ascontiguousarray(f_i.T.astype(np.float32, copy=False))
    if precision == "fp16":
        splits_v = (fTv.astype(np.float16),)
        splits_i = (fTi.astype(np.float16),)
        parts = ("hi",)
    else:
        splits_v = _hi_lo(fTv)
        splits_i = _hi_lo(fTi)
        parts = ("hi", "lo")
    c_lo = np.zeros(N_CORES, np.int64)
    in_maps = []
    for d in range(N_CORES):
        lmin = labels[d * RPC]
        lmax = labels[(d + 1) * RPC - 1]
        lo = int(np.searchsorted(labels, lmin, "left"))
        hi = int(np.searchsorted(labels, lmax, "right"))
        assert hi - lo <= W, f"band span {hi - lo} exceeds W={W}"
        c_lo[d] = lo
        sl = slice(d * RPC, (d + 1) * RPC)
        m = {
            "lab_loc": labels[sl].astype(np.float32),
            "lab_band": np.roll(labels, -lo)[:W].astype(np.float32)[None, :],
        }
        for pi, pn in enumerate(parts):
            m[f"lhs_v_{pn}"] = np.ascontiguousarray(splits_v[pi][:, sl])
            m[f"lhs_i_{pn}"] = np.ascontiguousarray(splits_i[pi][:, sl])
            m[f"rhs_v_{pn}"] = np.roll(splits_v[pi], -lo, axis=1)
            m[f"rhs_i_{pn}"] = np.roll(splits_i[pi], -lo, axis=1)
        in_maps.append(m)
    return in_maps, c_lo


def combine(results, labels, c_lo, W):
    """fp32 host combine in the reference's op order."""
    st = np.concatenate(
        [r["stats"].reshape(128, RT, 5).transpose(1, 0, 2).reshape(RPC, 5)
         for r in results], axis=0).astype(np.float32)
    num_v, rs_vv, ps_vv, rs_ii, ps_ii = (st[:, j] for j in range(5))
    num_i = np.zeros(B, np.float32)
    for d in range(N_CORES):
        np.add.at(num_i, (int(c_lo[d]) + np.arange(W)) % B,
                  results[d]["colsum"][0].astype(np.float32))
    counts = np.bincount(labels, minlength=int(labels.max()) + 1)
    gs = counts[labels].astype(np.float32)
    with np.errstate(all="ignore"):
        den_v = (rs_vv - ps_vv) + num_v
        den_i = (rs_ii - ps_ii) + num_i
        loss_v = np.mean(-np.log(num_v / den_v) / gs)
        loss_i = np.mean(-np.log(num_i / den_i) / gs)
    return np.asarray(np.float32(loss_v + loss_i))


def _pick_W(labels):
    span = 0
    for d in range(N_CORES):
        lmin = labels[d * RPC]
        lmax = labels[(d + 1) * RPC - 1]
        lo = int(np.searchsorted(labels, lmin, "left"))
        hi = int(np.searchsorted(labels, lmax, "right"))
        span = max(span, hi - lo)
    for W in (512, 640, 768, 896, 1024):
        if span <= W:
            return W
    raise ValueError(f"band span {span} exceeds max supported {QW}")


def run_kernel(f_v, f_i, labels, trace=False, trace_kwargs=None, precision: str = PRECISION):
    labels = np.asarray(labels).astype(np.int64, copy=False)
    f_v = np.asarray(f_v)
    f_i = np.asarray(f_i)
    assert f_v.shape == (B, D) and f_i.shape == (B, D) and labels.shape == (B,)
    if np.any(np.diff(labels) < 0):
        # the banded-mask scheme needs grouped labels; the loss is a mean
        # over rows, so a consistent row permutation leaves it unchanged
        order = np.argsort(labels, kind="stable")
        labels = labels[order]
        f_v = f_v[order]
        f_i = f_i[order]
    W = _pick_W(labels)
    nc = get_program(W, precision=precision)
    in_maps, c_lo = prepare_inputs(f_v, f_i, labels, W, precision=precision)
    res = run_bass_kernel_spmd(nc, in_maps, list(range(N_CORES)),
                               trace=trace, **(trace_kwargs or {}))
    loss = combine(res.results, labels, c_lo, W)
    return loss, res


def kernel(f_v, f_i, labels):
    loss, _ = run_kernel(f_v, f_i, labels)
    return loss


# revision 51
# speedup vs baseline: 1.0544x; 1.0544x over previous
"""Trainium2 Bass kernel for nn_ContrastiveLoss (B=4096, D=512, 8 cores).

Strategy (row-sharded, per the sharding hint):
  Each core owns 512 query rows.  It computes its row-blocks of the three
  similarity matrices S_vi, S_ii, S_vv as fp32 PE matmuls (lhsT = transposed
  local features, rhs = full transposed features with the *key axis rotated*
  per-core so that the same-identity column band sits at columns [0, W) for
  every core — this keeps the compiled program identical across cores).

  Per 128-row tile and matrix, the 4096-wide row lives in PSUM as four
  [128,1024] quarter tiles.  Each quarter gets its own row-max (DVE reduce,
  negated) so exp (ScalarE, fused row-sum via accum_out) can drain a quarter
  as soon as its own max is known — PSUM double-buffers, PE never stalls on
  the softmax tail.  Per-row fixup factors g_q = exp(m_q - max_q m_q)
  reconcile the per-quarter scales afterwards (cheap [128,4] ops).

  Masked (same-identity) sums only touch the W-wide band: mask built once
  per row tile from labels (is_equal against broadcast band labels), the
  band multiply runs on GpSimd, band row-sum on DVE.  The cross-core
  numerator of the i->v direction is a masked *column* sum: accumulated in
  SBUF across row tiles, reduced over partitions with a ones-vector PE
  matmul at the end, and all-reduced across cores on the host (it is the
  natural gather step — 8 x W floats).

  Host combine is fp32 in the reference's op order so fp32 degeneracies
  (underflow -> 0/0 -> NaN) reproduce faithfully.
"""

import sys

if "/opt/trn_rl_repo" not in sys.path:
    sys.path.insert(0, "/opt/trn_rl_repo")

from contextlib import ExitStack

import numpy as np

import concourse.bass as bass
import concourse.tile as tile
from concourse import mybir
from concourse.bass_utils import run_bass_kernel_spmd

F32 = mybir.dt.float32
BF16 = mybir.dt.bfloat16
FP16 = mybir.dt.float16
AX = mybir.AxisListType
ALU = mybir.AluOpType
ACTF = mybir.ActivationFunctionType

# "fp16": single-pass fp16 matmuls.  Final loss error ~2e-4 (per-row S error
#   ~1e-2 absolute averages down 64x in the 4096-row mean).
# "fp32": 3-pass hi/lo bf16 matmuls, S accurate to ~5e-5 (loss err ~1e-5) at
#   3x the PE cost.
PRECISION = "fp16"

B = 4096          # batch (rows of f_v / f_i)
D = 512           # feature dim
N_CORES = 8
RPC = B // N_CORES          # rows per core = 512
RT = RPC // 128             # row tiles per core = 4
KT = D // 128               # contraction k-tiles = 4
QW = 1024                   # PSUM quarter width (2 banks)
NQ = B // QW                # quarters per row = 4
CH = 512                    # matmul N-chunk (one PSUM bank)

_prog_cache: dict = {}


# --------------------------------------------------------------------------
# BIR legalization: this container's walrus encodes exactly one sem-wait and
# one sem-update per TPB instruction; Tile emits several.  Hoist extras onto
# adjacent single-wait/-update InstEventSemaphore instructions.
# --------------------------------------------------------------------------
_SPLIT_ID = [0]


def _legalize_syncs(nc, strip_final_barrier=True):
    if strip_final_barrier:
        # The Tile epilogue is: drain-all, all-engine barrier, semaphore
        # reset (InstISA on Pool), second all-engine barrier.  The second
        # barrier only orders engine halt vs nothing — execution completes
        # when all queues drain regardless, and the reset still runs before
        # the NEFF can be re-executed.  Dropping it saves ~3-4us of tail.
        for f in nc.m.functions:
            for blk in f.blocks:
                if not blk.name.endswith("_end"):
                    continue
                insts = list(blk.instructions)
                isa_idx = max((i for i, ins in enumerate(insts)
                               if type(ins).__name__ == "InstISA"), default=None)
                if isa_idx is not None and isa_idx < len(insts) - 1:
                    while len(blk.instructions) > isa_idx + 1:
                        blk.instructions.pop()
    for f in nc.m.functions:
        for blk in f.blocks:
            insts = list(blk.instructions)
            out = []
            changed = False
            for ins in insts:
                si = ins.sync_info
                if si is None:
                    out.append(ins)
                    continue
                waits = list(si.on_wait or [])
                updates = list(si.on_update or [])
                pre, post = [], []
                if len(waits) > 1:
                    changed = True
                    for w in waits[:-1]:
                        _SPLIT_ID[0] += 1
                        pre.append(mybir.InstEventSemaphore(
                            name=f"WSPLIT-{_SPLIT_ID[0]}", engine=ins.engine,
                            ins=[], outs=[],
                            sync_info=mybir.SyncInfo(on_wait=[w], on_update=[])))
                    waits = waits[-1:]
                if len(updates) > 1:
                    assert "DMA" not in type(ins).__name__, (
                        f"cannot split updates on DMA inst {ins.name}")
                    changed = True
                    for u in updates[1:]:
                        _SPLIT_ID[0] += 1
                        post.append(mybir.InstEventSemaphore(
                            name=f"USPLIT-{_SPLIT_ID[0]}", engine=ins.engine,
                            ins=[], outs=[],
                            sync_info=mybir.SyncInfo(on_wait=[], on_update=[u])))
                    updates = updates[:1]
                if pre or post:
                    ins.sync_info = mybir.SyncInfo(on_wait=waits, on_update=updates)
                out.extend(pre)
                out.append(ins)
                out.extend(post)
            if changed:
                while len(blk.instructions):
                    blk.instructions.pop()
                for ins in out:
                    blk.instructions.append(ins)


# --------------------------------------------------------------------------
# Device program
# --------------------------------------------------------------------------
def build_program(W: int, legalize: bool = True, precision: str = PRECISION) -> bass.Bass:
    """One SPMD program, identical across cores; W = masked band width."""
    assert W <= QW and W % 128 == 0 and W >= CH
    nc = bass.Bass()

    # fp32 PE matmuls lower to two half-rate passes on TRN2 (FP32HI/LO) —
    # 4x the cost of 16-bit.  Use 16-bit operands instead: either a single
    # fp16 pass, or a 3-pass hi/lo bf16 split (fp32-level accuracy).
    if precision == "fp16":
        parts, mm_dt = ("hi",), FP16
    else:
        parts, mm_dt = ("hi", "lo"), BF16
    feat = {}
    for nm in ("lhs_v", "lhs_i", "rhs_i", "rhs_v"):
        shape = [D, RPC] if nm.startswith("lhs") else [D, B]
        feat[nm] = tuple(
            nc.declare_dram_parameter(f"{nm}_{p}", shape, mm_dt, isOutput=False)
            for p in parts)
    lab_loc = nc.declare_dram_parameter("lab_loc", [RPC], F32, isOutput=False)
    lab_band = nc.declare_dram_parameter("lab_band", [1, W], F32, isOutput=False)
    stats = nc.declare_dram_parameter("stats", [128, RT * 5], F32, isOutput=True)
    colsum = nc.declare_dram_parameter("colsum", [1, W], F32, isOutput=True)

    with ExitStack() as ctx:
        tc = ctx.enter_context(tile.TileContext(nc))
        const = ctx.enter_context(tc.tile_pool(name="const", bufs=1))
        lhsp = ctx.enter_context(tc.tile_pool(name="lhsp", bufs=1))
        rhsp = ctx.enter_context(tc.tile_pool(name="rhsp", bufs=1))
        ebandp = ctx.enter_context(tc.tile_pool(name="ebandp", bufs=3))
        scrp = ctx.enter_context(tc.tile_pool(name="scrp", bufs=2))
        mbandp = ctx.enter_context(tc.tile_pool(name="mbandp", bufs=2))
        smallp = ctx.enter_context(tc.tile_pool(name="smallp", bufs=4))
        outp = ctx.enter_context(tc.tile_pool(name="outp", bufs=1))
        psum = ctx.enter_context(tc.tile_pool(name="psum", bufs=4, space="PSUM"))

        # ---- feature tiles (DMA order = first-needed first) ----
        # 1) lhs_v + the first column group of rhs_i gate the very first
        #    matmul; 2) labels/masks are needed ~25us in; 3) the rest.
        lhs_sb = {}
        rhs_sb = {}

        def lhs_dma(key, per_kt=False):
            pair = []
            for pi, pn in enumerate(parts):
                t_ = lhsp.tile([128, KT, RPC], mm_dt, tag=f"lhs{key}{pn}",
                               name=f"lhs{key}{pn}")
                pair.append(t_)
                src = feat[f"lhs_{key}"][pi][:, :].rearrange("(kt p) m -> p kt m", p=128)
                if per_kt:
                    for kt in range(KT):
                        nc.sync.dma_start(out=t_[:, kt, :], in_=src[:, kt, :])
                else:
                    nc.sync.dma_start(out=t_, in_=src)
            lhs_sb[key] = pair

        def rhs_alloc(key):
            rhs_sb[key] = [rhsp.tile([128, KT, B], mm_dt, tag=f"rhs{key}{pn}",
                                     name=f"rhs{key}{pn}")
                           for pn in parts]

        def rhs_dma(key, cg, kts=None):
            for pi in range(len(parts)):
                dram = feat[f"rhs_{key}"][pi]
                t_ = rhs_sb[key][pi]
                for kt in (range(KT) if kts is None else kts):
                    nc.sync.dma_start(
                        out=t_[:, kt, cg * 1024:(cg + 1) * 1024],
                        in_=dram[kt * 128:(kt + 1) * 128, cg * 1024:(cg + 1) * 1024])

        rhs_alloc("i")
        rhs_alloc("v")
        lhs_dma("v")
        rhs_dma("i", 0)
        rhs_dma("i", 1)

        lab_loc_sb = const.tile([128, RT], F32)
        nc.sync.dma_start(out=lab_loc_sb,
                          in_=lab_loc[:].rearrange("(t p) -> p t", p=128))
        lab_band_bc = const.tile([128, W], F32)
        lb = lab_band[:, :]
        nc.sync.dma_start(
            out=lab_band_bc,
            in_=bass.AP(tensor=lb.tensor, offset=lb.offset, ap=[[0, 128]] + list(lb.ap)[1:]),
        )

        rhs_dma("i", 2)
        rhs_dma("i", 3)

        ones = const.tile([128, 1], BF16)
        nc.vector.memset(ones, 1.0)

        # NB: tensor_scalar with an AP scalar lowers to TensorScalarPtr,
        # which measures ~9.5us per op on this silicon — use tensor_tensor
        # with stride-0 broadcast APs instead everywhere.
        masks = []
        for t in range(RT):
            m = const.tile([128, W], BF16, tag=f"mask{t}")
            masks.append(m)
            nc.vector.tensor_tensor(out=m, in0=lab_band_bc,
                                    in1=lab_loc_sb[:, t:t + 1].broadcast_to((128, W)),
                                    op=ALU.is_equal)

        lhs_dma("i")
        for cg in range(4):
            rhs_dma("v", cg)

        # ---- outputs / accumulators ----
        outstats = outp.tile([128, RT * 5], F32)
        macc = outp.tile([128, W], BF16)
        colsum_sb = outp.tile([1, W], F32)

        # exp segment layout: one ScalarE activation per (quarter x band/scr
        # region).  ScalarE reads up to the full [128, QW] PSUM quarter in one
        # op; the only split points are the band edge W (different dst) and a
        # 512 cap on band writes into e_band.
        seg_list = []          # (quarter, kind, lo, hi, accum_col)
        acc_col = 0
        for q in range(NQ):
            qlo, qhi = q * QW, (q + 1) * QW
            bounds = sorted({qlo, qhi, min(max(W, qlo), qhi)})
            for lo, hi in zip(bounds[:-1], bounds[1:]):
                kind = "band" if hi <= W else "scr"
                seg_list.append((q, kind, lo, hi, acc_col))
                acc_col += 1
        n_acc = acc_col
        # accum columns per quarter (for row-sum reconstruction)
        q_cols = [[s[4] for s in seg_list if s[0] == q] for q in range(NQ)]

        phases = (("vi", "v", "i"), ("ii", "i", "i"), ("vv", "v", "v"))
        for mname, lk, rk in phases:
            for t in range(RT):
                nh = smallp.tile([128, NQ], F32, tag="nh")
                rs = smallp.tile([128, n_acc], F32, tag="rs")
                e_band = ebandp.tile([128, W], BF16, tag="eband")
                ps_tiles = []
                # hi/lo passes; (hi,hi),(hi,lo) share the stationary
                # operand so weight loads amortize
                passes = ((0, 0),) if precision == "fp16" else ((0, 0), (0, 1), (1, 0))
                for q in range(NQ):
                    ps = psum.tile([128, QW], F32, tag="ps")
                    ps_tiles.append(ps)
                    for kt in range(KT):
                        for pidx, (pl, pr) in enumerate(passes):
                            for cq in range(QW // CH):
                                c = q * (QW // CH) + cq
                                nc.tensor.matmul(
                                    ps[:, cq * CH:(cq + 1) * CH],
                                    lhsT=lhs_sb[lk][pl][:, kt, t * 128:(t + 1) * 128],
                                    rhs=rhs_sb[rk][pr][:, kt, c * CH:(c + 1) * CH],
                                    start=(kt == 0 and pidx == 0),
                                    stop=(kt == KT - 1 and pidx == len(passes) - 1))
                    # negated quarter row-max
                    nc.vector.tensor_reduce(out=nh[:, q:q + 1], in_=ps, axis=AX.X,
                                            op=ALU.max, negate=True)

                # exp + fused row-sums, draining PSUM quarter by quarter
                # (urgent: frees PSUM slots for the next row-tile's matmuls)
                for q, kind, lo, hi, acol in seg_list:
                    src = ps_tiles[q][:, lo - q * QW:hi - q * QW]
                    if kind == "band":
                        dst = e_band[:, lo:hi]
                    else:
                        scr = scrp.tile([128, QW], F32, tag="scr")
                        dst = scr[:, :hi - lo]
                    nc.scalar.activation(out=dst, in_=src, func=ACTF.Exp,
                                         bias=nh[:, q:q + 1], scale=1.0,
                                         accum_out=rs[:, acol:acol + 1])

                # everything below only feeds the small per-row statistics;
                # schedule it as if emitted a row-tile later so it never
                # delays the next row-tile's maxes/exps on DVE/ACT
                with tc.high_priority(offset=-64):
                    # fixup factors g_q = exp(m_q - m),  nh = -m_q
                    # gtmp = nh - mmneg = m - m_q;  g4 = exp(-gtmp)
                    mmneg = smallp.tile([128, 1], F32, tag="mmneg")
                    nc.vector.tensor_reduce(out=mmneg, in_=nh, axis=AX.X, op=ALU.min)
                    gtmp = smallp.tile([128, NQ], F32, tag="gtmp")
                    nc.vector.tensor_tensor(out=gtmp, in0=nh,
                                            in1=mmneg.broadcast_to((128, NQ)),
                                            op=ALU.subtract)
                    g4 = smallp.tile([128, NQ], F32, tag="g4")
                    nc.scalar.activation(out=g4, in_=gtmp, func=ACTF.Exp, scale=-1.0)

                    # full row-sum (not needed for the vi matrix):
                    # sum_q g_q * (sum of that quarter's segment row-sums)
                    if mname != "vi":
                        gr = smallp.tile([128, NQ], F32, tag="gr")
                        # W <= QW so only quarter 0 can have two segments
                        c0 = q_cols[0]
                        if len(c0) > 1:
                            nc.vector.tensor_reduce(out=gr[:, 0:1],
                                                    in_=rs[:, c0[0]:c0[-1] + 1],
                                                    axis=AX.X, op=ALU.add)
                            nc.vector.tensor_tensor(out=gr[:, 0:1], in0=gr[:, 0:1],
                                                    in1=g4[:, 0:1], op=ALU.mult)
                            nc.vector.tensor_tensor(out=gr[:, 1:NQ], in0=g4[:, 1:NQ],
                                                    in1=rs[:, c0[-1] + 1:n_acc], op=ALU.mult)
                        else:
                            nc.vector.tensor_tensor(out=gr, in0=g4, in1=rs[:, 0:n_acc],
                                                    op=ALU.mult)
                        rs_col = {"ii": 3, "vv": 1}[mname]
                        nc.vector.tensor_reduce(out=outstats[:, t * 5 + rs_col:t * 5 + rs_col + 1],
                                                in_=gr, axis=AX.X, op=ALU.add)

                    # masked band sums (band in quarter 0 -> scale by g4[:,0])
                    masked = mbandp.tile([128, W], BF16, tag="masked")
                    nc.vector.tensor_tensor(out=masked, in0=e_band, in1=masks[t], op=ALU.mult)
                    praw = smallp.tile([128, 1], F32, tag="praw")
                    nc.vector.tensor_reduce(out=praw, in_=masked, axis=AX.X, op=ALU.add)
                    ps_col = {"vi": 0, "vv": 2, "ii": 4}[mname]
                    nc.vector.tensor_tensor(out=outstats[:, t * 5 + ps_col:t * 5 + ps_col + 1],
                                            in0=praw, in1=g4[:, 0:1], op=ALU.mult)

                    if mname == "vi":
                        # accumulate final-scale masked band for the column sums
                        g0b = g4[:, 0:1].broadcast_to((128, W))
                        if t == 0:
                            nc.vector.tensor_tensor(out=macc, in0=masked, in1=g0b,
                                                    op=ALU.mult)
                        else:
                            msc = mbandp.tile([128, W], BF16, tag="msc")
                            nc.vector.tensor_tensor(out=msc, in0=masked, in1=g0b,
                                                    op=ALU.mult)
                            nc.vector.tensor_tensor(out=macc, in0=macc, in1=msc, op=ALU.add)

        # ---- column-sum over the 512 local rows via ones-vector matmuls ----
        cps = psum.tile([128, QW], F32, tag="ps")
        nc.tensor.matmul(cps[0:1, 0:CH], lhsT=ones, rhs=macc[:, 0:CH],
                         start=True, stop=True)
        if W > CH:
            nc.tensor.matmul(cps[0:1, CH:W], lhsT=ones, rhs=macc[:, CH:W],
                             start=True, stop=True)
        nc.scalar.copy(out=colsum_sb, in_=cps[0:1, 0:W])
        nc.sync.dma_start(out=colsum[:, :], in_=colsum_sb)
        nc.sync.dma_start(out=stats[:, :], in_=outstats)

    if legalize:
        _legalize_syncs(nc)
    return nc


def get_program(W: int, legalize: bool = True, precision: str = PRECISION) -> bass.Bass:
    key = (W, legalize, precision)
    if key not in _prog_cache:
        _prog_cache[key] = build_program(W, legalize=legalize, precision=precision)
    return _prog_cache[key]


# --------------------------------------------------------------------------
# Host side
# --------------------------------------------------------------------------
def _hi_lo(x):
    """bf16 hi/lo split: x == hi + lo to ~2^-18 relative."""
    import ml_dtypes
    bf = ml_dtypes.bfloat16
    hi = x.astype(bf)
    lo = (x - hi.astype(np.float32)).astype(bf)
    return hi, lo


def prepare_inputs(f_v, f_i, labels, W, precision: str = PRECISION):
    """Returns (in_maps, c_lo) for the 8 cores."""
    fTv = np.ascontiguousarray(f_v.T.astype(np.float32, copy=False))
    fTi = np.ascontiguousarray(f_i.T.astype(np.float32, copy=False))
    if precision == "fp16":
        splits_v = (fTv.astype(np.float16),)
        splits_i = (fTi.astype(np.float16),)
        parts = ("hi",)
    else:
        splits_v = _hi_lo(fTv)
        splits_i = _hi_lo(fTi)
        parts = ("hi", "lo")
    c_lo = np.zeros(N_CORES, np.int64)
    in_maps = []
    for d in range(N_CORES):
        lmin = labels[d * RPC]
        lmax = labels[(d + 1) * RPC - 1]
        lo = int(np.searchsorted(labels, lmin, "left"))
        hi = int(np.searchsorted(labels, lmax, "right"))
        assert hi - lo <= W, f"band span {hi - lo} exceeds W={W}"
        c_lo[d] = lo
        sl = slice(d * RPC, (d + 1) * RPC)
        m = {
            "lab_loc": labels[sl].astype(np.float32),
            "lab_band": np.roll(labels, -lo)[:W].astype(np.float32)[None, :],
        }
        for pi, pn in enumerate(parts):
            m[f"lhs_v_{pn}"] = np.ascontiguousarray(splits_v[pi][:, sl])
            m[f"lhs_i_{pn}"] = np.ascontiguousarray(splits_i[pi][:, sl])
            m[f"rhs_v_{pn}"] = np.roll(splits_v[pi], -lo, axis=1)
            m[f"rhs_i_{pn}"] = np.roll(splits_i[pi], -lo, axis=1)
        in_maps.append(m)
    return in_maps, c_lo


def combine(results, labels, c_lo, W):
    """fp32 host combine in the reference's op order."""
    st = np.concatenate(
        [r["stats"].reshape(128, RT, 5).transpose(1, 0, 2).reshape(RPC, 5)
         for r in results], axis=0).astype(np.float32)
    num_v, rs_vv, ps_vv, rs_ii, ps_ii = (st[:, j] for j in range(5))
    num_i = np.zeros(B, np.float32)
    for d in range(N_CORES):
        np.add.at(num_i, (int(c_lo[d]) + np.arange(W)) % B,
                  results[d]["colsum"][0].astype(np.float32))
    counts = np.bincount(labels, minlength=int(labels.max()) + 1)
    gs = counts[labels].astype(np.float32)
    with np.errstate(all="ignore"):
        den_v = (rs_vv - ps_vv) + num_v
        den_i = (rs_ii - ps_ii) + num_i
        loss_v = np.mean(-np.log(num_v / den_v) / gs)
        loss_i = np.mean(-np.log(num_i / den_i) / gs)
    return np.asarray(np.float32(loss_v + loss_i))


def _pick_W(labels):
    span = 0
    for d in range(N_CORES):
        lmin = labels[d * RPC]
        lmax = labels[(d + 1) * RPC - 1]
        lo = int(np.searchsorted(labels, lmin, "left"))
        hi = int(np.searchsorted(labels, lmax, "right"))
        span = max(span, hi - lo)
    for W in (512, 640, 768, 896, 1024):
        if span <= W:
            return W
    raise ValueError(f"band span {span} exceeds max supported {QW}")


def run_kernel(f_v, f_i, labels, trace=False, trace_kwargs=None, precision: str = PRECISION):
    labels = np.asarray(labels).astype(np.int64, copy=False)
    f_v = np.asarray(f_v)
    f_i = np.asarray(f_i)
    assert f_v.shape == (B, D) and f_i.shape == (B, D) and labels.shape == (B,)
    if np.any(np.diff(labels) < 0):
        # the banded-mask scheme needs grouped labels; the loss is a mean
        # over rows, so a consistent row permutation leaves it unchanged
        order = np.argsort(labels, kind="stable")
        labels = labels[order]
        f_v = f_v[order]
        f_i = f_i[order]
    W = _pick_W(labels)
    nc = get_program(W, precision=precision)
    in_maps, c_lo = prepare_inputs(f_v, f_i, labels, W, precision=precision)
    res = run_bass_kernel_spmd(nc, in_maps, list(range(N_CORES)),
                               trace=trace, **(trace_kwargs or {}))
    loss = combine(res.results, labels, c_lo, W)
    return loss, res


def kernel(f_v, f_i, labels):
    loss, _ = run_kernel(f_v, f_i, labels)
    return loss
